# revision 7
# baseline (speedup 1.0000x reference)
"""Discriminator-loss kernel for Trainium2, SPMD across 8 NeuronCores.

Computes mean(where(s == other_s, 1, -1) * x) for N = 2^25 elements.

Strategy (data-parallel, per the sharding hint): each core streams its
1/8 shard of (s, other_s, x) from HBM and reduces it with two DVE ops
per tile:
    eq   = is_equal(s, other_s)            # int32 -> f32 {0.0, 1.0}
    prod = (eq - 0.5) * x                  # = +-x/2, exact in f32
    acc[:, t] = sum_freeaxis(prod)         # fused accum of the same op
Per-core output is the [128, T] grid of partial sums of (+-x/2); the
host sums the 8*128*T partials in float64 and multiplies by 2/N.
"""

import contextlib
import ctypes
import os
import sys
import types

import numpy as np


def _install_ntff_hook_shim():
    """Register the axon NTFF-profile hook if the image's ``antenv`` lacks
    ``axon_hooks`` (boot degrades silently in that case, which breaks
    ``run_bass_kernel_spmd(trace=True)``). Same ctypes recipe as
    ``trn_agent_boot.trn_boot._ntff_profile_via_ctypes``. No-op when the
    module already exists or the .so is absent."""
    try:
        import antenv.axon_hooks  # noqa: F401

        return
    except ImportError:
        pass
    try:
        mod = types.ModuleType("antenv.axon_hooks")
        holder = {"hook": None}
        mod.set_axon_ntff_profile_hook = lambda h: holder.__setitem__("hook", h)
        mod.get_axon_ntff_profile_hook = lambda: holder["hook"]
        sys.modules["antenv.axon_hooks"] = mod
        try:
            import antenv

            antenv.axon_hooks = mod
        except ImportError:
            pass

        so_path = "/opt/axon/libaxon_pjrt.so"
        if not os.path.exists(so_path):
            return
        lib = ctypes.CDLL(so_path)
        if not hasattr(lib, "axon_start_nrt_profile"):
            return
        lib.axon_start_nrt_profile.argtypes = [
            ctypes.POINTER(ctypes.c_int64),
            ctypes.c_size_t,
        ]
        lib.axon_start_nrt_profile.restype = ctypes.c_int64
        lib.axon_stop_nrt_profile.argtypes = [ctypes.c_char_p]
        lib.axon_stop_nrt_profile.restype = ctypes.c_int64

        @contextlib.contextmanager
        def _hook(output_dir, device_ids):
            import jax

            jax.devices()
            if device_ids:
                ids = (ctypes.c_int64 * len(device_ids))(*device_ids)
                rc = lib.axon_start_nrt_profile(ids, len(device_ids))
            else:
                rc = lib.axon_start_nrt_profile(None, 0)
            if rc != 0:
                raise RuntimeError(f"axon_start_nrt_profile rc={rc}")
            try:
                yield
            finally:
                n = lib.axon_stop_nrt_profile(str(output_dir).encode())
                print(f"ntff profile: {n} file(s) -> {output_dir}", file=sys.stderr)

        holder["hook"] = _hook
    except Exception:
        pass


_install_ntff_hook_shim()

from concourse import bacc, mybir, tile
from concourse.bass_utils import run_bass_kernel_spmd

N = 33554432
NCORES = 8
PER = N // NCORES  # 4194304 elements per core
P = 128            # SBUF partitions
F = 4096           # free elements per DMA tile (2 MiB f32 tiles)
T = PER // (P * F)  # 8 tiles per tensor per core
FC = 1024          # compute sub-tile (keeps the post-last-DMA tail short)
NSUB = F // FC

_cache = {}


def _build():
    if "nc" in _cache:
        return _cache["nc"]

    nc = bacc.Bacc(
        "TRN2", target_bir_lowering=False, debug=False, num_devices=NCORES
    )

    s_in = nc.dram_tensor("s", [T, P, F], mybir.dt.int32, kind="ExternalInput")
    o_in = nc.dram_tensor(
        "other_s", [T, P, F], mybir.dt.int32, kind="ExternalInput"
    )
    x_in = nc.dram_tensor("x", [T, P, F], mybir.dt.float32, kind="ExternalInput")
    out = nc.dram_tensor(
        "out", [P, T * NSUB], mybir.dt.float32, kind="ExternalOutput"
    )

    with tile.TileContext(nc) as tc:
        with (
            tc.tile_pool(name="io", bufs=3) as io_pool,
            tc.tile_pool(name="work", bufs=2) as work_pool,
            tc.tile_pool(name="stat", bufs=1) as stat_pool,
        ):
            acc = stat_pool.tile([P, T * NSUB], mybir.dt.float32)
            for t in range(T):
                s_t = io_pool.tile([P, F], mybir.dt.int32, tag="s")
                o_t = io_pool.tile([P, F], mybir.dt.int32, tag="o")
                x_t = io_pool.tile([P, F], mybir.dt.float32, tag="x")
                nc.sync.dma_start(out=s_t[:], in_=s_in[t])
                nc.sync.dma_start(out=o_t[:], in_=o_in[t])
                nc.sync.dma_start(out=x_t[:], in_=x_in[t])
                for j in range(NSUB):
                    sl = slice(j * FC, (j + 1) * FC)
                    eq = work_pool.tile([P, FC], mybir.dt.float32, tag="eq")
                    nc.vector.tensor_tensor(
                        out=eq[:],
                        in0=s_t[:, sl],
                        in1=o_t[:, sl],
                        op=mybir.AluOpType.is_equal,
                    )
                    nc.vector.scalar_tensor_tensor(
                        out=eq[:],
                        in0=eq[:],
                        scalar=-0.5,
                        in1=x_t[:, sl],
                        op0=mybir.AluOpType.add,
                        op1=mybir.AluOpType.mult,
                        accum_out=acc[:, t * NSUB + j : t * NSUB + j + 1],
                    )
            nc.sync.dma_start(out=out[:], in_=acc[:])

    nc.compile()
    _cache["nc"] = nc
    return nc


def _shard(a, c):
    return np.ascontiguousarray(a[c * PER : (c + 1) * PER]).reshape(T, P, F)


def run(s, other_s, x, **spmd_kwargs):
    """Run on HW; returns (full_output, BassKernelResults)."""
    s = np.ascontiguousarray(np.asarray(s, dtype=np.int32).reshape(N))
    other_s = np.ascontiguousarray(np.asarray(other_s, dtype=np.int32).reshape(N))
    x = np.ascontiguousarray(np.asarray(x, dtype=np.float32).reshape(N))

    nc = _build()
    in_maps = [
        {"s": _shard(s, c), "other_s": _shard(other_s, c), "x": _shard(x, c)}
        for c in range(NCORES)
    ]
    res = run_bass_kernel_spmd(nc, in_maps, core_ids=list(range(NCORES)), **spmd_kwargs)

    total = 0.0
    for r in res.results:
        total += float(np.sum(r["out"].astype(np.float64)))
    full = np.array(2.0 * total / N, dtype=np.float32)
    return full, res


def kernel(s, other_s, x):
    out, _ = run(s, other_s, x)
    return out


# revision 9
# speedup vs baseline: 1.0170x; 1.0170x over previous
"""Discriminator-loss kernel for Trainium2, SPMD across 8 NeuronCores.

Computes mean(where(s == other_s, 1, -1) * x) for N = 2^25 elements.

Strategy (data-parallel, per the sharding hint): each core streams its
1/8 shard of (s, other_s, x) from HBM and reduces it with two DVE ops
per tile:
    eq   = is_equal(s, other_s)            # int32 -> f32 {0.0, 1.0}
    prod = (eq - 0.5) * x                  # = +-x/2, exact in f32
    acc[:, t] = sum_freeaxis(prod)         # fused accum of the same op
Per-core output is the [128, T] grid of partial sums of (+-x/2); the
host sums the 8*128*T partials in float64 and multiplies by 2/N.
"""

import contextlib
import ctypes
import os
import sys
import types

import numpy as np


def _install_ntff_hook_shim():
    """Register the axon NTFF-profile hook if the image's ``antenv`` lacks
    ``axon_hooks`` (boot degrades silently in that case, which breaks
    ``run_bass_kernel_spmd(trace=True)``). Same ctypes recipe as
    ``trn_agent_boot.trn_boot._ntff_profile_via_ctypes``. No-op when the
    module already exists or the .so is absent."""
    try:
        import antenv.axon_hooks  # noqa: F401

        return
    except ImportError:
        pass
    try:
        mod = types.ModuleType("antenv.axon_hooks")
        holder = {"hook": None}
        mod.set_axon_ntff_profile_hook = lambda h: holder.__setitem__("hook", h)
        mod.get_axon_ntff_profile_hook = lambda: holder["hook"]
        sys.modules["antenv.axon_hooks"] = mod
        try:
            import antenv

            antenv.axon_hooks = mod
        except ImportError:
            pass

        so_path = "/opt/axon/libaxon_pjrt.so"
        if not os.path.exists(so_path):
            return
        lib = ctypes.CDLL(so_path)
        if not hasattr(lib, "axon_start_nrt_profile"):
            return
        lib.axon_start_nrt_profile.argtypes = [
            ctypes.POINTER(ctypes.c_int64),
            ctypes.c_size_t,
        ]
        lib.axon_start_nrt_profile.restype = ctypes.c_int64
        lib.axon_stop_nrt_profile.argtypes = [ctypes.c_char_p]
        lib.axon_stop_nrt_profile.restype = ctypes.c_int64

        @contextlib.contextmanager
        def _hook(output_dir, device_ids):
            import jax

            jax.devices()
            if device_ids:
                ids = (ctypes.c_int64 * len(device_ids))(*device_ids)
                rc = lib.axon_start_nrt_profile(ids, len(device_ids))
            else:
                rc = lib.axon_start_nrt_profile(None, 0)
            if rc != 0:
                raise RuntimeError(f"axon_start_nrt_profile rc={rc}")
            try:
                yield
            finally:
                n = lib.axon_stop_nrt_profile(str(output_dir).encode())
                print(f"ntff profile: {n} file(s) -> {output_dir}", file=sys.stderr)

        holder["hook"] = _hook
    except Exception:
        pass


_install_ntff_hook_shim()

from concourse import bacc, mybir, tile
from concourse.bass_utils import run_bass_kernel_spmd

N = 33554432
NCORES = 8
PER = N // NCORES  # 4194304 elements per core
P = 128            # SBUF partitions
F = 4096           # free elements per DMA tile (2 MiB f32 tiles)
T = PER // (P * F)  # 8 tiles per tensor per core
FC = 1024          # compute sub-tile (keeps the post-last-DMA tail short)
NSUB = F // FC

_cache = {}


def _build():
    if "nc" in _cache:
        return _cache["nc"]

    nc = bacc.Bacc(
        "TRN2", target_bir_lowering=False, debug=False, num_devices=NCORES
    )

    s_in = nc.dram_tensor("s", [T, P, F], mybir.dt.int32, kind="ExternalInput")
    o_in = nc.dram_tensor(
        "other_s", [T, P, F], mybir.dt.int32, kind="ExternalInput"
    )
    x_in = nc.dram_tensor("x", [T, P, F], mybir.dt.float32, kind="ExternalInput")
    out = nc.dram_tensor(
        "out", [P, T * NSUB], mybir.dt.float32, kind="ExternalOutput"
    )

    with tile.TileContext(nc) as tc:
        with (
            tc.tile_pool(name="io", bufs=2) as io_pool,
            tc.tile_pool(name="edge", bufs=6) as edge_pool,
            tc.tile_pool(name="work", bufs=2) as work_pool,
            tc.tile_pool(name="stat", bufs=1) as stat_pool,
        ):
            acc = stat_pool.tile([P, T * NSUB], mybir.dt.float32)

            def compute(s_ap, o_ap, x_ap, col):
                eq = work_pool.tile([P, FC], mybir.dt.float32, tag="eq")
                nc.vector.tensor_tensor(
                    out=eq[:], in0=s_ap, in1=o_ap, op=mybir.AluOpType.is_equal
                )
                nc.vector.scalar_tensor_tensor(
                    out=eq[:],
                    in0=eq[:],
                    scalar=-0.5,
                    in1=x_ap,
                    op0=mybir.AluOpType.add,
                    op1=mybir.AluOpType.mult,
                    accum_out=acc[:, col : col + 1],
                )

            for t in range(T):
                if t == 0 or t == T - 1:
                    # Tapered edge tiles: 512 KiB sub-DMAs so the pipeline
                    # fills fast at the start and the last compute quantum
                    # gates on a small DMA at the end.
                    for j in range(NSUB):
                        sl = slice(j * FC, (j + 1) * FC)
                        s_t = edge_pool.tile([P, FC], mybir.dt.int32, tag="se")
                        o_t = edge_pool.tile([P, FC], mybir.dt.int32, tag="oe")
                        x_t = edge_pool.tile([P, FC], mybir.dt.float32, tag="xe")
                        nc.sync.dma_start(out=s_t[:], in_=s_in[t][:, sl])
                        nc.sync.dma_start(out=o_t[:], in_=o_in[t][:, sl])
                        nc.sync.dma_start(out=x_t[:], in_=x_in[t][:, sl])
                        compute(s_t[:], o_t[:], x_t[:], t * NSUB + j)
                else:
                    s_t = io_pool.tile([P, F], mybir.dt.int32, tag="s")
                    o_t = io_pool.tile([P, F], mybir.dt.int32, tag="o")
                    x_t = io_pool.tile([P, F], mybir.dt.float32, tag="x")
                    nc.sync.dma_start(out=s_t[:], in_=s_in[t])
                    nc.sync.dma_start(out=o_t[:], in_=o_in[t])
                    nc.sync.dma_start(out=x_t[:], in_=x_in[t])
                    for j in range(NSUB):
                        sl = slice(j * FC, (j + 1) * FC)
                        compute(
                            s_t[:, sl], o_t[:, sl], x_t[:, sl], t * NSUB + j
                        )
            nc.sync.dma_start(out=out[:], in_=acc[:])

    nc.compile()
    _cache["nc"] = nc
    return nc


def _shard(a, c):
    return np.ascontiguousarray(a[c * PER : (c + 1) * PER]).reshape(T, P, F)


def run(s, other_s, x, **spmd_kwargs):
    """Run on HW; returns (full_output, BassKernelResults)."""
    s = np.ascontiguousarray(np.asarray(s, dtype=np.int32).reshape(N))
    other_s = np.ascontiguousarray(np.asarray(other_s, dtype=np.int32).reshape(N))
    x = np.ascontiguousarray(np.asarray(x, dtype=np.float32).reshape(N))

    nc = _build()
    in_maps = [
        {"s": _shard(s, c), "other_s": _shard(other_s, c), "x": _shard(x, c)}
        for c in range(NCORES)
    ]
    res = run_bass_kernel_spmd(nc, in_maps, core_ids=list(range(NCORES)), **spmd_kwargs)

    total = 0.0
    for r in res.results:
        total += float(np.sum(r["out"].astype(np.float64)))
    full = np.array(2.0 * total / N, dtype=np.float32)
    return full, res


def kernel(s, other_s, x):
    out, _ = run(s, other_s, x)
    return out


# revision 11
# speedup vs baseline: 1.0661x; 1.0483x over previous
"""Discriminator-loss kernel for Trainium2, SPMD across 8 NeuronCores.

Computes mean(where(s == other_s, 1, -1) * x) for N = 2^25 elements.

Strategy (data-parallel, per the sharding hint): each core streams its
1/8 shard of (s, other_s, x) from HBM and reduces it with two DVE ops
per tile:
    eq   = is_equal(s, other_s)            # int32 -> f32 {0.0, 1.0}
    prod = (eq - 0.5) * x                  # = +-x/2, exact in f32
    acc[:, t] = sum_freeaxis(prod)         # fused accum of the same op
Per-core output is the [128, T] grid of partial sums of (+-x/2); the
host sums the 8*128*T partials in float64 and multiplies by 2/N.
"""

import contextlib
import ctypes
import os
import sys
import types

import numpy as np


def _install_ntff_hook_shim():
    """Register the axon NTFF-profile hook if the image's ``antenv`` lacks
    ``axon_hooks`` (boot degrades silently in that case, which breaks
    ``run_bass_kernel_spmd(trace=True)``). Same ctypes recipe as
    ``trn_agent_boot.trn_boot._ntff_profile_via_ctypes``. No-op when the
    module already exists or the .so is absent."""
    try:
        import antenv.axon_hooks  # noqa: F401

        return
    except ImportError:
        pass
    try:
        mod = types.ModuleType("antenv.axon_hooks")
        holder = {"hook": None}
        mod.set_axon_ntff_profile_hook = lambda h: holder.__setitem__("hook", h)
        mod.get_axon_ntff_profile_hook = lambda: holder["hook"]
        sys.modules["antenv.axon_hooks"] = mod
        try:
            import antenv

            antenv.axon_hooks = mod
        except ImportError:
            pass

        so_path = "/opt/axon/libaxon_pjrt.so"
        if not os.path.exists(so_path):
            return
        lib = ctypes.CDLL(so_path)
        if not hasattr(lib, "axon_start_nrt_profile"):
            return
        lib.axon_start_nrt_profile.argtypes = [
            ctypes.POINTER(ctypes.c_int64),
            ctypes.c_size_t,
        ]
        lib.axon_start_nrt_profile.restype = ctypes.c_int64
        lib.axon_stop_nrt_profile.argtypes = [ctypes.c_char_p]
        lib.axon_stop_nrt_profile.restype = ctypes.c_int64

        @contextlib.contextmanager
        def _hook(output_dir, device_ids):
            import jax

            jax.devices()
            if device_ids:
                ids = (ctypes.c_int64 * len(device_ids))(*device_ids)
                rc = lib.axon_start_nrt_profile(ids, len(device_ids))
            else:
                rc = lib.axon_start_nrt_profile(None, 0)
            if rc != 0:
                raise RuntimeError(f"axon_start_nrt_profile rc={rc}")
            try:
                yield
            finally:
                n = lib.axon_stop_nrt_profile(str(output_dir).encode())
                print(f"ntff profile: {n} file(s) -> {output_dir}", file=sys.stderr)

        holder["hook"] = _hook
    except Exception:
        pass


_install_ntff_hook_shim()

from concourse import bacc, mybir, tile
from concourse.bass_utils import run_bass_kernel_spmd

N = 33554432
NCORES = 8
PER = N // NCORES  # 4194304 elements per core
P = 128            # SBUF partitions
F = 4096           # free elements per DMA tile (2 MiB f32 tiles)
T = PER // (P * F)  # 8 tiles per tensor per core
FC = 1024          # compute sub-tile (keeps the post-last-DMA tail short)
NSUB = F // FC

_cache = {}


class _FastExitTileContext(tile.TileContext):
    """TileContext with a lighter kernel exit.

    The stock exit emits drain + a wait on every (sem, tick) the kernel
    produced (split into EVENT_SEMAPHORE chains, ~7 us for this kernel) +
    barrier + per-sem clear + barrier. The entry preamble of the next
    execution re-clears all semaphores anyway, and `all_engine_barrier`
    already emits per-engine DRAINs (flushing the DMA queues, including the
    final output DMA on sync). So: one drain+barrier, no clears.
    """

    def _drain_and_barrier(self, tick_clock, wait_clock):
        popped = self.nc._tile_sem_poison_stack.pop()
        assert popped is self._sem_poison
        self.nc.all_engine_barrier()


def _build():
    if "nc" in _cache:
        return _cache["nc"]

    nc = bacc.Bacc(
        "TRN2", target_bir_lowering=False, debug=False, num_devices=NCORES
    )

    s_in = nc.dram_tensor("s", [T, P, F], mybir.dt.int32, kind="ExternalInput")
    o_in = nc.dram_tensor(
        "other_s", [T, P, F], mybir.dt.int32, kind="ExternalInput"
    )
    x_in = nc.dram_tensor("x", [T, P, F], mybir.dt.float32, kind="ExternalInput")
    out = nc.dram_tensor(
        "out", [P, T * NSUB], mybir.dt.float32, kind="ExternalOutput"
    )

    with _FastExitTileContext(nc) as tc:
        with (
            tc.tile_pool(name="io", bufs=2) as io_pool,
            tc.tile_pool(name="edge", bufs=6) as edge_pool,
            tc.tile_pool(name="work", bufs=2) as work_pool,
            tc.tile_pool(name="stat", bufs=1) as stat_pool,
        ):
            acc = stat_pool.tile([P, T * NSUB], mybir.dt.float32)

            def compute(s_ap, o_ap, x_ap, col):
                eq = work_pool.tile([P, FC], mybir.dt.float32, tag="eq")
                nc.vector.tensor_tensor(
                    out=eq[:], in0=s_ap, in1=o_ap, op=mybir.AluOpType.is_equal
                )
                nc.vector.scalar_tensor_tensor(
                    out=eq[:],
                    in0=eq[:],
                    scalar=-0.5,
                    in1=x_ap,
                    op0=mybir.AluOpType.add,
                    op1=mybir.AluOpType.mult,
                    accum_out=acc[:, col : col + 1],
                )

            for t in range(T):
                if t == 0 or t == T - 1:
                    # Tapered edge tiles: 512 KiB sub-DMAs so the pipeline
                    # fills fast at the start and the last compute quantum
                    # gates on a small DMA at the end.
                    for j in range(NSUB):
                        sl = slice(j * FC, (j + 1) * FC)
                        s_t = edge_pool.tile([P, FC], mybir.dt.int32, tag="se")
                        o_t = edge_pool.tile([P, FC], mybir.dt.int32, tag="oe")
                        x_t = edge_pool.tile([P, FC], mybir.dt.float32, tag="xe")
                        nc.sync.dma_start(out=s_t[:], in_=s_in[t][:, sl])
                        nc.sync.dma_start(out=o_t[:], in_=o_in[t][:, sl])
                        nc.sync.dma_start(out=x_t[:], in_=x_in[t][:, sl])
                        compute(s_t[:], o_t[:], x_t[:], t * NSUB + j)
                else:
                    s_t = io_pool.tile([P, F], mybir.dt.int32, tag="s")
                    o_t = io_pool.tile([P, F], mybir.dt.int32, tag="o")
                    x_t = io_pool.tile([P, F], mybir.dt.float32, tag="x")
                    nc.sync.dma_start(out=s_t[:], in_=s_in[t])
                    nc.sync.dma_start(out=o_t[:], in_=o_in[t])
                    nc.sync.dma_start(out=x_t[:], in_=x_in[t])
                    for j in range(NSUB):
                        sl = slice(j * FC, (j + 1) * FC)
                        compute(
                            s_t[:, sl], o_t[:, sl], x_t[:, sl], t * NSUB + j
                        )
            nc.sync.dma_start(out=out[:], in_=acc[:])

    nc.compile()
    _cache["nc"] = nc
    return nc


def _shard(a, c):
    return np.ascontiguousarray(a[c * PER : (c + 1) * PER]).reshape(T, P, F)


def run(s, other_s, x, **spmd_kwargs):
    """Run on HW; returns (full_output, BassKernelResults)."""
    s = np.ascontiguousarray(np.asarray(s, dtype=np.int32).reshape(N))
    other_s = np.ascontiguousarray(np.asarray(other_s, dtype=np.int32).reshape(N))
    x = np.ascontiguousarray(np.asarray(x, dtype=np.float32).reshape(N))

    nc = _build()
    in_maps = [
        {"s": _shard(s, c), "other_s": _shard(other_s, c), "x": _shard(x, c)}
        for c in range(NCORES)
    ]
    res = run_bass_kernel_spmd(nc, in_maps, core_ids=list(range(NCORES)), **spmd_kwargs)

    total = 0.0
    for r in res.results:
        total += float(np.sum(r["out"].astype(np.float64)))
    full = np.array(2.0 * total / N, dtype=np.float32)
    return full, res


def kernel(s, other_s, x):
    out, _ = run(s, other_s, x)
    return out
